# revision 19
# baseline (speedup 1.0000x reference)
"""Trainium2 kernel for nn_HEAnsatz: 21-qubit hardware-efficient ansatz.

Circuit structure: RY-layer, CNOT-chain, RY-layer, CNOT-chain, RY-layer on
|0...0>.  All gates are real, and the CNOT chain is a nearest-neighbor
staircase, so the final state is exactly a bond-dimension-4 matrix product
state.  Splitting the 21 qubits 11/10 gives the full statevector as a rank-4
outer product

    state.reshape(2048, 1024) = L @ R.T,   L: (2048, 4), R: (1024, 4)

L and R are built on host in fp64 (O(10^5) flops); the 2^21-element
expansion — the actual memory-bound work — runs on 8 NeuronCores: core i
computes rows [256*i, 256*(i+1)) of L @ R.T and streams the 1 MiB f32 shard
to HBM.

On-device the rank-4 contraction runs on the tensor engine as a K=16 bf16
matmul: L and R are split into exact bf16 hi+lo pairs (L = Lhi + Llo), and
the 16 contraction rows enumerate (a, u, v) in {4 bond} x {hi,lo} x {hi,lo}
with lhsT row = Lu_a and rhs row = Rv_a, so PSUM accumulates
sum_a (Lhi_a+Llo_a)(Rhi_a+Rlo_a) in fp32 — full product at bf16 matmul
speed, rel err ~3e-6 instead of ~2e-3 for plain bf16.
"""

import numpy as np

N_QUBITS = 21
N_CORES = 8
ROWS_PER_CORE = 2048 // N_CORES  # 256
N_COLS = 1024


def _build_LR(params: np.ndarray):
    """Build the rank-4 factor matrices L (2048,4), R (1024,4) in fp64."""
    p = params.astype(np.float64)
    c1, s1 = np.cos(p[0:21] * 0.5), np.sin(p[0:21] * 0.5)
    c2, s2 = np.cos(p[21:42] * 0.5), np.sin(p[21:42] * 0.5)
    c3, s3 = np.cos(p[42:63] * 0.5), np.sin(p[42:63] * 0.5)

    # Site transfer tensor: A[k, y, (w', x'), (w, x)] = R3[y,w] R2[w^w', x] u[x^x']
    # with u = (c1, s1) the RY1|0> column, bond = (prev CNOT-layer-2 bit w',
    # prev CNOT-layer-1 bit x').
    A = np.empty((N_QUBITS, 2, 4, 4), dtype=np.float64)
    for k in range(N_QUBITS):
        R2 = np.array([[c2[k], -s2[k]], [s2[k], c2[k]]])
        R3 = np.array([[c3[k], -s3[k]], [s3[k], c3[k]]])
        u = np.array([c1[k], s1[k]])
        for y in range(2):
            for wp in range(2):
                for xp in range(2):
                    for w in range(2):
                        for x in range(2):
                            A[k, y, wp * 2 + xp, w * 2 + x] = (
                                R3[y, w] * R2[w ^ wp, x] * u[x ^ xp]
                            )

    # Left boundary: bits w'(-1) = x'(-1) = 0  ->  row e_{(0,0)}.
    V = np.zeros((1, 4))
    V[0, 0] = 1.0
    for k in range(11):  # qubits 0..10 -> 2048 prefixes
        V = np.einsum("pa,yab->pyb", V, A[k]).reshape(-1, 4)
    # Right boundary: free sum over the final bond -> ones.
    W = np.ones((1, 4))
    for k in range(N_QUBITS - 1, 10, -1):  # qubits 20..11 -> 1024 suffixes
        W = np.einsum("yab,tb->yta", A[k], W).reshape(-1, 4)
    return V, W  # (2048, 4), (1024, 4)


def _pack_bf16_k16(L: np.ndarray, R: np.ndarray):
    """Pack hi/lo-split factors into the K=16 lhsT (16,2048) / rhs (16,1024)."""
    import ml_dtypes

    bf16 = ml_dtypes.bfloat16
    Lhi = L.astype(bf16)
    Llo = (L - Lhi.astype(np.float64)).astype(bf16)
    Rhi = R.astype(bf16)
    Rlo = (R - Rhi.astype(np.float64)).astype(bf16)

    lhsT = np.empty((16, L.shape[0]), dtype=bf16)
    rhs = np.empty((16, R.shape[0]), dtype=bf16)
    k = 0
    for a in range(4):
        for Lu in (Lhi, Llo):
            for Rv in (Rhi, Rlo):
                lhsT[k] = Lu[:, a]
                rhs[k] = Rv[:, a]
                k += 1
    return lhsT, rhs


_NC_CACHE = {}


def _build_bass():
    """Per-core kernel: out(256,1024) f32 = lhsT.T @ rhs with K=16 bf16 inputs.

    Input packed as one (16, 1280) bf16 tensor: cols 0:256 = lhsT shard
    (256 output rows), cols 256:1280 = rhs (1024 output cols).
    Four (128, 512) chunks pipelined: PE matmul -> DVE copy -> HWDGE DMA out,
    stores split across the SP and ACT HWDGE rings.
    """
    import concourse.bass as bass
    import concourse.mybir as mybir

    # Bass.__init__ unconditionally emits const-AP memsets plus an
    # all-engine barrier before any user instruction; this kernel uses no
    # const APs, and the ~2us barrier would gate the input DMA. Suppress
    # both during construction only.
    orig_barrier = bass.Bass.all_engine_barrier
    bass.Bass.all_engine_barrier = lambda self, **kw: None
    orig_gp_memset = bass.BassGpSimd.memset
    bass.BassGpSimd.memset = lambda self, *a, **kw: None
    try:
        nc = bass.Bass()
    finally:
        bass.Bass.all_engine_barrier = orig_barrier
        bass.BassGpSimd.memset = orig_gp_memset
    f32 = mybir.dt.float32
    bf16 = mybir.dt.bfloat16

    lr = nc.dram_tensor("lr", [16, 1280], bf16, kind="ExternalInput")
    out = nc.dram_tensor("out", [ROWS_PER_CORE, N_COLS], f32, kind="ExternalOutput")

    with (
        nc.sbuf_tensor("lr_sb", [16, 1280], bf16) as lr_sb,
        nc.sbuf_tensor("out_sb", [128, 2048], f32) as out_sb,
        nc.sbuf_tensor("warm_sb", [128, 8], f32) as warm_sb,
        nc.sbuf_tensor("warm_in", [16, 640], bf16) as warm_in,
        nc.psum_tensor("ps0", [128, 1024], f32) as ps0,
        nc.psum_tensor("ps1", [128, 1024], f32) as ps1,
        nc.psum_tensor("ps_warm", [128, 512], f32) as ps_warm,
        nc.semaphore("dma_sem") as dma_sem,
        nc.semaphore("dmab_sem") as dmab_sem,
        nc.semaphore("dma2_sem") as dma2_sem,
        nc.semaphore("mm_sem") as mm_sem,
        nc.semaphore("cp_sem") as cp_sem,
        nc.semaphore("cp2_sem") as cp2_sem,
    ):
        lt0 = lr_sb[:, 0:128]
        lt1 = lr_sb[:, 128:256]
        r0 = lr_sb[:, 256:768]
        r1 = lr_sb[:, 768:1280]

        # ACT: input load first (ACT reaches the body earliest), split so the
        # lhsT + N0 half wakes the PE a little sooner; warm the activation
        # table under the DMA latency.  Chunk naming: c0=(M0,N0), c1=(M0,N1),
        # c2=(M1,N0), c3=(M1,N1); PE runs N0 halves first.
        nc.scalar.dma_start(out=lr_sb[:, 0:768], in_=lr[:, 0:768]).then_inc(dma_sem, 16)
        nc.scalar.dma_start(out=lr_sb[:, 768:1280], in_=lr[:, 768:1280]).then_inc(
            dmab_sem, 16
        )
        nc.scalar.copy(warm_sb[:], warm_sb[:])
        # ACT copies c2 and c3 back-to-back (a dma_start between them would
        # occupy the ACT sequencer ~0.6us), then issues c1's and c3's stores
        # on its HWDGE ring.
        nc.scalar.wait_ge(mm_sem, 2)
        nc.scalar.copy(out_sb[:, 1024:1536], ps1[:, 0:512]).then_inc(cp2_sem, 1)
        nc.scalar.wait_ge(mm_sem, 4)
        nc.scalar.copy(out_sb[:, 1536:2048], ps1[:, 512:1024]).then_inc(cp2_sem, 1)
        nc.scalar.wait_ge(cp_sem, 2)
        nc.scalar.dma_start(out=out[0:128, 512:1024], in_=out_sb[:, 512:1024]).then_inc(
            dma2_sem, 16
        )
        nc.scalar.wait_ge(cp2_sem, 2)
        nc.scalar.dma_start(
            out=out[128:256, 512:1024], in_=out_sb[:, 1536:2048]
        ).then_inc(dma2_sem, 16)

        # SP: store chunks 0 and 2 on the SP HWDGE ring
        nc.sync.wait_ge(cp_sem, 1)
        nc.sync.dma_start(out=out[0:128, 0:512], in_=out_sb[:, 0:512]).then_inc(
            dma_sem, 16
        )
        nc.sync.wait_ge(cp2_sem, 1)
        nc.sync.dma_start(out=out[128:256, 0:512], in_=out_sb[:, 1024:1536]).then_inc(
            dma_sem, 16
        )

        # PE: 4 matmuls (one PSUM bank each), N0 halves first
        nc.tensor.wait_ge(dma_sem, 16)
        nc.tensor.matmul(ps0[:, 0:512], lt0, r0, start=True, stop=True).then_inc(
            mm_sem, 1
        )
        nc.tensor.matmul(ps1[:, 0:512], lt1, r0, start=True, stop=True).then_inc(
            mm_sem, 1
        )
        nc.tensor.wait_ge(dmab_sem, 16)
        nc.tensor.matmul(ps0[:, 512:1024], lt0, r1, start=True, stop=True).then_inc(
            mm_sem, 1
        )
        nc.tensor.matmul(ps1[:, 512:1024], lt1, r1, start=True, stop=True).then_inc(
            mm_sem, 1
        )

        # DVE: copy chunks 0 and 1
        nc.vector.wait_ge(mm_sem, 1)
        nc.vector.tensor_copy(out_sb[:, 0:512], ps0[:, 0:512]).then_inc(cp_sem, 1)
        nc.vector.wait_ge(mm_sem, 3)
        nc.vector.tensor_copy(out_sb[:, 512:1024], ps0[:, 512:1024]).then_inc(cp_sem, 1)

    return nc


def kernel(params: np.ndarray) -> np.ndarray:
    from concourse.bass_utils import run_bass_kernel_spmd

    L, R = _build_LR(np.asarray(params))
    lhsT, rhs = _pack_bf16_k16(L, R)  # (16, 2048), (16, 1024) bf16

    in_maps = []
    for i in range(N_CORES):
        packed = np.empty((16, 1280), dtype=lhsT.dtype)
        packed[:, 0:ROWS_PER_CORE] = lhsT[
            :, i * ROWS_PER_CORE : (i + 1) * ROWS_PER_CORE
        ]
        packed[:, ROWS_PER_CORE:] = rhs
        in_maps.append({"lr": packed})

    if "nc" not in _NC_CACHE:
        _NC_CACHE["nc"] = _build_bass()
    nc = _NC_CACHE["nc"]

    res = run_bass_kernel_spmd(nc, in_maps, list(range(N_CORES)))
    shards = [res.results[i]["out"] for i in range(N_CORES)]
    full = np.concatenate(shards, axis=0).reshape(-1)  # (2**21,) f32
    return full.astype(np.complex128)


# revision 20
# speedup vs baseline: 1.0377x; 1.0377x over previous
"""Trainium2 kernel for nn_HEAnsatz: 21-qubit hardware-efficient ansatz.

Circuit structure: RY-layer, CNOT-chain, RY-layer, CNOT-chain, RY-layer on
|0...0>.  All gates are real, and the CNOT chain is a nearest-neighbor
staircase, so the final state is exactly a bond-dimension-4 matrix product
state.  Splitting the 21 qubits 11/10 gives the full statevector as a rank-4
outer product

    state.reshape(2048, 1024) = L @ R.T,   L: (2048, 4), R: (1024, 4)

L and R are built on host in fp64 (O(10^5) flops); the 2^21-element
expansion — the actual memory-bound work — runs on 8 NeuronCores: core i
computes rows [256*i, 256*(i+1)) of L @ R.T and streams the 1 MiB f32 shard
to HBM.

On-device the rank-4 contraction runs on the tensor engine as a K=16 bf16
matmul: L and R are split into exact bf16 hi+lo pairs (L = Lhi + Llo), and
the 16 contraction rows enumerate (a, u, v) in {4 bond} x {hi,lo} x {hi,lo}
with lhsT row = Lu_a and rhs row = Rv_a, so PSUM accumulates
sum_a (Lhi_a+Llo_a)(Rhi_a+Rlo_a) in fp32 — full product at bf16 matmul
speed, rel err ~3e-6 instead of ~2e-3 for plain bf16.
"""

import numpy as np

N_QUBITS = 21
N_CORES = 8
ROWS_PER_CORE = 2048 // N_CORES  # 256
N_COLS = 1024


def _build_LR(params: np.ndarray):
    """Build the rank-4 factor matrices L (2048,4), R (1024,4) in fp64."""
    p = params.astype(np.float64)
    c1, s1 = np.cos(p[0:21] * 0.5), np.sin(p[0:21] * 0.5)
    c2, s2 = np.cos(p[21:42] * 0.5), np.sin(p[21:42] * 0.5)
    c3, s3 = np.cos(p[42:63] * 0.5), np.sin(p[42:63] * 0.5)

    # Site transfer tensor: A[k, y, (w', x'), (w, x)] = R3[y,w] R2[w^w', x] u[x^x']
    # with u = (c1, s1) the RY1|0> column, bond = (prev CNOT-layer-2 bit w',
    # prev CNOT-layer-1 bit x').
    A = np.empty((N_QUBITS, 2, 4, 4), dtype=np.float64)
    for k in range(N_QUBITS):
        R2 = np.array([[c2[k], -s2[k]], [s2[k], c2[k]]])
        R3 = np.array([[c3[k], -s3[k]], [s3[k], c3[k]]])
        u = np.array([c1[k], s1[k]])
        for y in range(2):
            for wp in range(2):
                for xp in range(2):
                    for w in range(2):
                        for x in range(2):
                            A[k, y, wp * 2 + xp, w * 2 + x] = (
                                R3[y, w] * R2[w ^ wp, x] * u[x ^ xp]
                            )

    # Left boundary: bits w'(-1) = x'(-1) = 0  ->  row e_{(0,0)}.
    V = np.zeros((1, 4))
    V[0, 0] = 1.0
    for k in range(11):  # qubits 0..10 -> 2048 prefixes
        V = np.einsum("pa,yab->pyb", V, A[k]).reshape(-1, 4)
    # Right boundary: free sum over the final bond -> ones.
    W = np.ones((1, 4))
    for k in range(N_QUBITS - 1, 10, -1):  # qubits 20..11 -> 1024 suffixes
        W = np.einsum("yab,tb->yta", A[k], W).reshape(-1, 4)
    return V, W  # (2048, 4), (1024, 4)


def _pack_bf16_k16(L: np.ndarray, R: np.ndarray):
    """Pack hi/lo-split factors into the K=16 lhsT (16,2048) / rhs (16,1024)."""
    import ml_dtypes

    bf16 = ml_dtypes.bfloat16
    Lhi = L.astype(bf16)
    Llo = (L - Lhi.astype(np.float64)).astype(bf16)
    Rhi = R.astype(bf16)
    Rlo = (R - Rhi.astype(np.float64)).astype(bf16)

    lhsT = np.empty((16, L.shape[0]), dtype=bf16)
    rhs = np.empty((16, R.shape[0]), dtype=bf16)
    k = 0
    for a in range(4):
        for Lu in (Lhi, Llo):
            for Rv in (Rhi, Rlo):
                lhsT[k] = Lu[:, a]
                rhs[k] = Rv[:, a]
                k += 1
    return lhsT, rhs


_NC_CACHE = {}


def _build_bass():
    """Per-core kernel: out(256,1024) f32 = lhsT.T @ rhs with K=16 bf16 inputs.

    Input packed as one (16, 1280) bf16 tensor: cols 0:256 = lhsT shard
    (256 output rows), cols 256:1280 = rhs (1024 output cols).
    Four (128, 512) chunks pipelined: PE matmul -> DVE copy -> HWDGE DMA out,
    stores split across the SP and ACT HWDGE rings.
    """
    import concourse.bass as bass
    import concourse.mybir as mybir

    # Bass.__init__ unconditionally emits const-AP memsets plus an
    # all-engine barrier before any user instruction; this kernel uses no
    # const APs, and the ~2us barrier would gate the input DMA. Suppress
    # both during construction only.
    orig_barrier = bass.Bass.all_engine_barrier
    bass.Bass.all_engine_barrier = lambda self, **kw: None
    orig_gp_memset = bass.BassGpSimd.memset
    bass.BassGpSimd.memset = lambda self, *a, **kw: None
    try:
        nc = bass.Bass()
    finally:
        bass.Bass.all_engine_barrier = orig_barrier
        bass.BassGpSimd.memset = orig_gp_memset
    f32 = mybir.dt.float32
    bf16 = mybir.dt.bfloat16

    lr = nc.dram_tensor("lr", [16, 1280], bf16, kind="ExternalInput")
    out = nc.dram_tensor("out", [ROWS_PER_CORE, N_COLS], f32, kind="ExternalOutput")

    with (
        nc.sbuf_tensor("lr_sb", [16, 1280], bf16) as lr_sb,
        nc.sbuf_tensor("out_sb", [128, 2048], f32) as out_sb,
        nc.sbuf_tensor("warm_sb", [128, 8], f32) as warm_sb,
        nc.sbuf_tensor("warm_in", [16, 640], bf16) as warm_in,
        nc.psum_tensor("ps0", [128, 1024], f32) as ps0,
        nc.psum_tensor("ps1", [128, 1024], f32) as ps1,
        nc.psum_tensor("ps_warm", [128, 512], f32) as ps_warm,
        nc.semaphore("dma_sem") as dma_sem,
        nc.semaphore("dmab_sem") as dmab_sem,
        nc.semaphore("dma2_sem") as dma2_sem,
        nc.semaphore("mm_sem") as mm_sem,
        nc.semaphore("cp_sem") as cp_sem,
        nc.semaphore("cp2_sem") as cp2_sem,
    ):
        lt0 = lr_sb[:, 0:128]
        lt1 = lr_sb[:, 128:256]
        r0 = lr_sb[:, 256:768]
        r1 = lr_sb[:, 768:1280]

        # ACT: input load first (ACT reaches the body earliest), split so the
        # lhsT + N0 half wakes the PE a little sooner; warm the activation
        # table under the DMA latency.  Chunk naming: c0=(M0,N0), c1=(M0,N1),
        # c2=(M1,N0), c3=(M1,N1); PE runs N0 halves first.
        nc.scalar.dma_start(out=lr_sb[:, 0:768], in_=lr[:, 0:768]).then_inc(dma_sem, 16)
        nc.scalar.dma_start(out=lr_sb[:, 768:1280], in_=lr[:, 768:1280]).then_inc(
            dmab_sem, 16
        )
        nc.scalar.copy(warm_sb[:], warm_sb[:])
        # ACT copies c2 and c3 back-to-back, then self-issues only c3's store.
        # (A dma_start between them would occupy the ACT sequencer ~0.6us; the
        # self-wait after a retired copy is ~0.1us.)
        nc.scalar.wait_ge(mm_sem, 2)
        nc.scalar.copy(out_sb[:, 1024:1536], ps1[:, 0:512]).then_inc(cp2_sem, 1)
        nc.scalar.wait_ge(mm_sem, 4)
        nc.scalar.copy(out_sb[:, 1536:2048], ps1[:, 512:1024]).then_inc(cp2_sem, 1)
        nc.scalar.wait_ge(cp2_sem, 2)
        nc.scalar.dma_start(
            out=out[128:256, 512:1024], in_=out_sb[:, 1536:2048]
        ).then_inc(dma2_sem, 16)

        # GpSimd (SWDGE): store chunk 2, off the critical path
        nc.gpsimd.wait_ge(cp2_sem, 1)
        nc.gpsimd.dma_start(
            out=out[128:256, 0:512], in_=out_sb[:, 1024:1536]
        ).then_inc(dma2_sem, 16)

        # SP: store chunks 0 and 1 on the SP HWDGE ring
        nc.sync.wait_ge(cp_sem, 1)
        nc.sync.dma_start(out=out[0:128, 0:512], in_=out_sb[:, 0:512]).then_inc(
            dma_sem, 16
        )
        nc.sync.wait_ge(cp_sem, 2)
        nc.sync.dma_start(out=out[0:128, 512:1024], in_=out_sb[:, 512:1024]).then_inc(
            dma_sem, 16
        )

        # PE: 4 matmuls (one PSUM bank each), N0 halves first
        nc.tensor.wait_ge(dma_sem, 16)
        nc.tensor.matmul(ps0[:, 0:512], lt0, r0, start=True, stop=True).then_inc(
            mm_sem, 1
        )
        nc.tensor.matmul(ps1[:, 0:512], lt1, r0, start=True, stop=True).then_inc(
            mm_sem, 1
        )
        nc.tensor.wait_ge(dmab_sem, 16)
        nc.tensor.matmul(ps0[:, 512:1024], lt0, r1, start=True, stop=True).then_inc(
            mm_sem, 1
        )
        nc.tensor.matmul(ps1[:, 512:1024], lt1, r1, start=True, stop=True).then_inc(
            mm_sem, 1
        )

        # DVE: copy chunks 0 and 1
        nc.vector.wait_ge(mm_sem, 1)
        nc.vector.tensor_copy(out_sb[:, 0:512], ps0[:, 0:512]).then_inc(cp_sem, 1)
        nc.vector.wait_ge(mm_sem, 3)
        nc.vector.tensor_copy(out_sb[:, 512:1024], ps0[:, 512:1024]).then_inc(cp_sem, 1)

    return nc


def kernel(params: np.ndarray) -> np.ndarray:
    from concourse.bass_utils import run_bass_kernel_spmd

    L, R = _build_LR(np.asarray(params))
    lhsT, rhs = _pack_bf16_k16(L, R)  # (16, 2048), (16, 1024) bf16

    in_maps = []
    for i in range(N_CORES):
        packed = np.empty((16, 1280), dtype=lhsT.dtype)
        packed[:, 0:ROWS_PER_CORE] = lhsT[
            :, i * ROWS_PER_CORE : (i + 1) * ROWS_PER_CORE
        ]
        packed[:, ROWS_PER_CORE:] = rhs
        in_maps.append({"lr": packed})

    if "nc" not in _NC_CACHE:
        _NC_CACHE["nc"] = _build_bass()
    nc = _NC_CACHE["nc"]

    res = run_bass_kernel_spmd(nc, in_maps, list(range(N_CORES)))
    shards = [res.results[i]["out"] for i in range(N_CORES)]
    full = np.concatenate(shards, axis=0).reshape(-1)  # (2**21,) f32
    return full.astype(np.complex128)
